# revision 1
# baseline (speedup 1.0000x reference)
"""BiLSTM tagger kernel, direction-parallel sharding over 8 NeuronCores.

Core c in 0..3 runs the FORWARD direction for sequences [32c, 32c+32);
core c+4 runs the BACKWARD direction for the same sequences (inputs
pre-reversed per sequence length on the host). Every matmul therefore has
M=32 batch rows instead of 16 — recurrent matmul cost is N-bound, so this
halves per-core PE work per step — and each core runs ONE scan per layer,
halving ScalarE/VectorE contention.

Between layers, the forward/backward halves are exchanged with a pairwise
AllGather (replica groups {c, c+4}) of the layer's scan-order output into
hpair [2*ntok, H]: slot 0 = lower rank = forward data on BOTH cores, so
the program is identical across cores; all per-core asymmetry (reversal
indices, which weights, which token half of the classifier) lives in
host-computed input tensors. Gate order i,f,o,g (sigmoid block first),
all matmuls bf16, PSUM fp32, gx injected into PSUM via identity matmuls
one step ahead.
"""

import sys

for _p in ("/opt/trn_rl_repo",):
    if _p not in sys.path:
        sys.path.append(_p)

import numpy as np
import ml_dtypes

import concourse.bass as bass
import concourse.tile as tile
from concourse import bacc, mybir
from concourse.bass import IndirectOffsetOnAxis
from concourse.bass_utils import run_bass_kernel_spmd

F32 = mybir.dt.float32
BF16 = mybir.dt.bfloat16
I32 = mybir.dt.int32
AF = mybir.ActivationFunctionType
ALU = mybir.AluOpType

B, T, V, E, H, TAGS = 128, 512, 50000, 256, 512, 64
NC = 8
NPAIR = NC // 2          # 4 sequence groups
BL = B // NPAIR          # 32 sequences per core (one direction each)
G = 4 * H
GROUPS = [[c, c + NPAIR] for c in range(NPAIR)]

# gate order i,g,f,o: half A = (i,g) finishes first so t2 = sig(i)*tanh(g)
# overlaps half B's matmuls; half B = (f,o) needs only ONE sigmoid call
_GATE_PERM = np.concatenate([
    np.arange(0, H), np.arange(2 * H, 3 * H), np.arange(H, 2 * H),
    np.arange(3 * H, 4 * H)])


def _build(nc, Tn=T, Bl=BL, TC=2, RC=4):
    ntok = Bl * Tn            # 16384 per core
    nchunk = ntok // 128      # 128
    ncls = ntok // 2 // 128   # 64 classifier chunks (half the pair's tokens)
    KE = E // 128
    KH2 = 2 * H // 128
    KH = H // 128

    # ---- dram I/O (per-core data resolves fwd/bwd asymmetry) ----
    emb = nc.dram_tensor("emb", [V, E], F32, kind="ExternalInput")
    xg_idx = nc.dram_tensor("xg_idx", [128, nchunk], I32, kind="ExternalInput")
    iA2 = nc.dram_tensor("iA2", [128, nchunk], I32, kind="ExternalInput")
    iB2 = nc.dram_tensor("iB2", [128, nchunk], I32, kind="ExternalInput")
    icA = nc.dram_tensor("icA", [128, ncls], I32, kind="ExternalInput")
    icB = nc.dram_tensor("icB", [128, ncls], I32, kind="ExternalInput")
    mask = nc.dram_tensor("mask", [Bl, Tn], F32, kind="ExternalInput")
    ident = nc.dram_tensor("ident", [32, 32], BF16, kind="ExternalInput")

    wih, whh, biasd = {}, {}, {}
    for l, din in (("l1", E), ("l2", 2 * H)):
        wih[l] = nc.dram_tensor(f"wihT_{l}", [din, G], BF16, kind="ExternalInput")
        whh[l] = nc.dram_tensor(f"whhT_{l}", [H, G], BF16, kind="ExternalInput")
        biasd[l] = nc.dram_tensor(f"bias_{l}", [128, G], F32, kind="ExternalInput")
    wcls = nc.dram_tensor("wclsT", [2 * H, TAGS], BF16, kind="ExternalInput")
    bcls = nc.dram_tensor("bcls", [TAGS, 1], F32, kind="ExternalInput")

    gx = {l: nc.dram_tensor(f"gx_{l}", [ntok, G], BF16) for l in ("l1", "l2")}
    hloc = {l: nc.dram_tensor(f"hloc_{l}", [ntok, H], BF16) for l in ("l1", "l2")}
    hpair = {l: nc.dram_tensor(f"hpair_{l}", [2 * ntok, H], BF16)
             for l in ("l1", "l2")}
    logitsT = nc.dram_tensor("logitsT", [TAGS, ntok // 2], F32,
                             kind="ExternalOutput")

    with tile.TileContext(nc) as tc:
        with tc.tile_pool(name="const", bufs=1) as cpool:
            def load_const(nm, shape, dt, src_ap):
                t = cpool.tile(shape, dt, name=nm, tag=nm)
                nc.gpsimd.dma_start(t[:], src_ap)
                return t

            xg_sb = load_const("xg_sb", [128, nchunk], I32, xg_idx[:])
            iA2_sb = load_const("iA2_sb", [128, nchunk], I32, iA2[:])
            iB2_sb = load_const("iB2_sb", [128, nchunk], I32, iB2[:])
            icA_sb = load_const("icA_sb", [128, ncls], I32, icA[:])
            icB_sb = load_const("icB_sb", [128, ncls], I32, icB[:])
            mask_sb = load_const("mask_sb", [Bl, Tn], F32, mask[:])
            id_sb = load_const("id_sb", [32, 32], BF16, ident[:])
            bcls_sb = load_const("bcls_sb", [TAGS, 1], F32, bcls[:])
            bias_sb = {l: load_const(f"bias_sb_{l}", [128, G], F32, biasd[l][:])
                       for l in ("l1", "l2")}
            wcls_sb = cpool.tile([128, KH2, TAGS], BF16, name="wcls_sb")
            for k in range(KH2):
                nc.gpsimd.dma_start(wcls_sb[:, k, :], wcls[128 * k:128 * (k + 1), :])

            # layer-1 proj (emb gather) + scan + exchange
            _proj(nc, tc, nchunk, KE, wih["l1"], bias_sb["l1"], gx["l1"],
                  emb, xg_sb, None, None, is_emb=True)
            _scan(nc, tc, Tn, Bl, TC, RC, KH, whh["l1"], gx["l1"], hloc["l1"],
                  mask_sb, id_sb)
            nc.gpsimd.collective_compute(
                "AllGather", ALU.bypass, GROUPS,
                ins=[hloc["l1"][:]], outs=[hpair["l1"][:]])
            # layer-2 proj (gathers from hpair) + scan + exchange
            _proj(nc, tc, nchunk, KH2, wih["l2"], bias_sb["l2"], gx["l2"],
                  hpair["l1"], None, iA2_sb, iB2_sb, is_emb=False)
            _scan(nc, tc, Tn, Bl, TC, RC, KH, whh["l2"], gx["l2"], hloc["l2"],
                  mask_sb, id_sb)
            nc.gpsimd.collective_compute(
                "AllGather", ALU.bypass, GROUPS,
                ins=[hloc["l2"][:]], outs=[hpair["l2"][:]])

            # classifier over this core's half of the pair's tokens
            with tc.tile_pool(name="cls", bufs=3) as gp, \
                 tc.tile_pool(name="clsT", bufs=3) as gtp, \
                 tc.tile_pool(name="clsps", bufs=4, space="PSUM") as pp, \
                 tc.tile_pool(name="clso", bufs=3) as op:
                for c in range(ncls):
                    o2 = gp.tile([128, 2 * H], BF16, tag="in")
                    nc.gpsimd.indirect_dma_start(
                        out=o2[:, 0:H], out_offset=None, in_=hpair["l2"][:],
                        in_offset=IndirectOffsetOnAxis(ap=icA_sb[:, c:c + 1], axis=0))
                    nc.gpsimd.indirect_dma_start(
                        out=o2[:, H:2 * H], out_offset=None, in_=hpair["l2"][:],
                        in_offset=IndirectOffsetOnAxis(ap=icB_sb[:, c:c + 1], axis=0))
                    o2T = gtp.tile([128, KH2, 128], BF16, tag="inT")
                    for k in range(KH2):
                        nc.sync.dma_start_transpose(
                            o2T[:, k, :], o2[:, 128 * k:128 * (k + 1)])
                    ps = pp.tile([TAGS, 128], F32, name="clsps_t")
                    for k in range(KH2):
                        nc.tensor.matmul(ps[:], wcls_sb[:, k, :], o2T[:, k, :],
                                         start=(k == 0), stop=(k == KH2 - 1))
                    lg = op.tile([TAGS, 128], F32, tag="lg")
                    nc.scalar.activation(lg[:], ps[:], AF.Identity,
                                         bias=bcls_sb[:, 0:1])
                    nc.gpsimd.dma_start(logitsT[:, 128 * c:128 * (c + 1)], lg[:])
    return nc


def _proj(nc, tc, nchunk, KD, wih_d, bias_t, gx_d, src, emb_idx, iA, iB, is_emb):
    """gx = input @ W_ih^T + b in scan-time order. Layer 1: fp32 emb row
    gather + cast. Layer 2: two bf16 row gathers from hpair."""
    D = KD * 128
    G_ = G
    with tc.tile_pool(name="pw", bufs=1) as wpool, \
         tc.tile_pool(name="pg", bufs=3) as gpool, \
         tc.tile_pool(name="pgT", bufs=3) as tpool, \
         tc.tile_pool(name="pps", bufs=4, space="PSUM") as ppool, \
         tc.tile_pool(name="pout", bufs=3) as opool:
        wsb = wpool.tile([128, KD, G_], BF16, tag="w", name="wih_sb")
        for k in range(KD):
            nc.gpsimd.dma_start(wsb[:, k, :], wih_d[128 * k:128 * (k + 1), :])
        for c in range(nchunk):
            if is_emb:
                e32 = gpool.tile([128, D], F32, tag="e32")
                nc.gpsimd.indirect_dma_start(
                    out=e32[:], out_offset=None, in_=src[:],
                    in_offset=IndirectOffsetOnAxis(ap=emb_idx[:, c:c + 1], axis=0))
                xin = gpool.tile([128, D], BF16, tag="e16")
                nc.vector.tensor_copy(xin[:], e32[:])
            else:
                xin = gpool.tile([128, D], BF16, tag="e16")
                nc.gpsimd.indirect_dma_start(
                    out=xin[:, 0:H], out_offset=None, in_=src[:],
                    in_offset=IndirectOffsetOnAxis(ap=iA[:, c:c + 1], axis=0))
                nc.gpsimd.indirect_dma_start(
                    out=xin[:, H:2 * H], out_offset=None, in_=src[:],
                    in_offset=IndirectOffsetOnAxis(ap=iB[:, c:c + 1], axis=0))
            xT = tpool.tile([128, KD, 128], BF16, tag="xT")
            for k in range(KD):
                nc.sync.dma_start_transpose(xT[:, k, :], xin[:, 128 * k:128 * (k + 1)])
            gout = opool.tile([128, G_], BF16, tag="gout")
            for n in range(G_ // 512):
                ps = ppool.tile([128, 512], F32, name="pps")
                for k in range(KD):
                    nc.tensor.matmul(
                        ps[:], xT[:, k, :], wsb[:, k, 512 * n:512 * (n + 1)],
                        start=(k == 0), stop=(k == KD - 1))
                nc.vector.tensor_tensor(
                    out=gout[:, 512 * n:512 * (n + 1)], in0=ps[:],
                    in1=bias_t[:, 512 * n:512 * (n + 1)], op=ALU.add)
            nc.gpsimd.dma_start(gx_d[128 * c:128 * (c + 1), :], gout[:])


def _scan(nc, tc, Tn, Bl, TC, RC, KH, whh_d, gx_d, hout_d, mask_sb, id_sb):
    """Single-direction scan, M=32 batch. Same software-pipelined gx
    injection as the 2-scan variant, one scan per core."""
    gxv = gx_d.ap().rearrange("(b t) d -> b t d", b=Bl)
    houtv = hout_d.ap().rearrange("(b t) d -> b t d", b=Bl)
    H2 = 2 * H
    with tc.tile_pool(name="sw", bufs=1) as wpool, \
         tc.tile_pool(name="sgx", bufs=4) as gxpool, \
         tc.tile_pool(name="sst", bufs=1) as stpool, \
         tc.tile_pool(name="sps", bufs=4, space="PSUM") as pspool, \
         tc.tile_pool(name="swk", bufs=3) as wkpool, \
         tc.tile_pool(name="shT", bufs=3) as htpool, \
         tc.tile_pool(name="srng", bufs=3) as rpool:
        wsb = wpool.tile([128, KH, G], BF16, tag="whh", name="whh_sb")
        for k in range(KH):
            nc.gpsimd.dma_start(wsb[:, k, :], whh_d[128 * k:128 * (k + 1), :])
        hT = [htpool.tile([128, KH * Bl], BF16, tag="hT", name="hT0")]
        nc.vector.memset(hT[0][:], 0.0)
        c_st = stpool.tile([Bl, H], F32, tag="c", name="c_st")
        nc.vector.memset(c_st[:], 0.0)
        gxc = {}
        gA = [None]
        gB = [None]
        ring = [None]
        nwin = (Tn + TC - 1) // TC

        def load_gx(w):
            tl = gxpool.tile([Bl, TC, G], BF16, tag="gx", name="gxc")
            nc.gpsimd.dma_start(tl[:], gxv[:, w * TC:(w + 1) * TC, :])
            gxc[w] = tl
            gxc.pop(w - 3, None)

        def inject(tt):
            gA[0] = pspool.tile([Bl, H2], F32, tag="ps", name="gA")
            gB[0] = pspool.tile([Bl, H2], F32, tag="ps", name="gB")
            gxt = gxc[tt // TC]
            for half, lo in ((gA[0], 0), (gB[0], H2)):
                for n in range(2):
                    nc.tensor.matmul(
                        half[:, 512 * n:512 * (n + 1)], id_sb[:],
                        gxt[:, tt % TC, lo + 512 * n:lo + 512 * (n + 1)],
                        start=True, stop=False, skip_group_check=True)

        load_gx(0)
        if nwin > 1:
            load_gx(1)
        inject(0)
        for t in range(Tn):
            gAc, gBc = gA[0], gB[0]
            gact = wkpool.tile([Bl, G], F32, tag="gact", name="gact")
            t1 = wkpool.tile([Bl, H], F32, tag="t1", name="t1")
            t2 = wkpool.tile([Bl, H], F32, tag="t2", name="t2")
            if t % RC == 0:
                ring[0] = rpool.tile([Bl, RC, H], BF16, tag="ring", name="ring")
            for half, cols in ((gAc, (0, 1)), (gBc, (2, 3))):
                for n in cols:
                    dst_lo = 512 * (n % 2)
                    for k in range(KH):
                        nc.tensor.matmul(
                            half[:, dst_lo:dst_lo + 512],
                            hT[0][:, Bl * k:Bl * (k + 1)],
                            wsb[:, k, 512 * n:512 * (n + 1)],
                            start=False, stop=(k == KH - 1),
                            skip_group_check=True)
                if half is gAc:
                    # i,g activations + t2 run under half B's matmuls
                    nc.scalar.activation(gact[:, 0:H], gAc[:, 0:H], AF.Sigmoid)
                    nc.scalar.activation(gact[:, H:H2], gAc[:, H:H2], AF.Tanh)
                    nc.vector.tensor_tensor(out=t2[:], in0=gact[:, 0:H],
                                            in1=gact[:, H:H2], op=ALU.mult)
            # sig(f) depends only on half B's f-columns (n=2 chain), so it
            # fires ~0.9us before the o-columns finish; sig(o) is off the
            # c-path and runs while the DVE does t1/c
            nc.scalar.activation(gact[:, H2:3 * H], gBc[:, 0:H], AF.Sigmoid)
            nc.vector.tensor_tensor(out=t1[:], in0=gact[:, H2:3 * H],
                                    in1=c_st[:], op=ALU.mult)
            nc.vector.tensor_tensor(out=c_st[:], in0=t1[:], in1=t2[:], op=ALU.add)
            tch = wkpool.tile([Bl, H], F32, tag="tch", name="tch")
            # sig(o) before tanh(c) in the ScalarE FIFO: its input (the
            # o-columns' matmul chain) is ready before c is
            nc.scalar.activation(gact[:, 3 * H:G], gBc[:, H:H2], AF.Sigmoid)
            nc.scalar.activation(tch[:], c_st[:], AF.Tanh)
            h16 = wkpool.tile([Bl, H], BF16, tag="h16", name="h16")
            nc.vector.tensor_tensor(out=h16[:], in0=gact[:, 3 * H:G],
                                    in1=tch[:], op=ALU.mult)
            hT_ps = pspool.tile([128, KH * Bl], F32, tag="ps", name="hT_ps")
            if t + 1 < Tn:
                if (t + 1) % TC == 0 and (t + 1) // TC + 1 < nwin:
                    load_gx((t + 1) // TC + 1)
                inject(t + 1)
            # per-chunk copy right behind each transpose so next step's
            # first matmul (which reads only chunk 0) starts early
            hTn = htpool.tile([128, KH * Bl], BF16, tag="hT", name="hTn")
            for k in range(KH):
                nc.tensor.matmul(hT_ps[:, Bl * k:Bl * (k + 1)],
                                 h16[:, 128 * k:128 * (k + 1)], id_sb[:],
                                 start=True, stop=True)
                nc.scalar.activation(hTn[:, Bl * k:Bl * (k + 1)],
                                     hT_ps[:, Bl * k:Bl * (k + 1)], AF.Copy)
            hT[0] = hTn
            nc.vector.tensor_scalar_mul(ring[0][:, t % RC, :], h16[:],
                                        mask_sb[:, t:t + 1])
            if (t + 1) % RC == 0:
                t0r = t + 1 - RC
                nc.gpsimd.dma_start(houtv[:, t0r:t0r + RC, :], ring[0][:, :, :])


def _prep_inputs(inputs, Tn=T, Bl=BL):
    x = np.asarray(inputs["x"]).astype(np.int32)
    lengths = np.asarray(inputs["lengths"]).astype(np.int32)
    emb = np.asarray(inputs["emb"], dtype=np.float32)
    ntok = Bl * Tn
    bf = ml_dtypes.bfloat16

    wt = {}
    for s in ("f1", "b1", "f2", "b2"):
        w_ih = np.asarray(inputs[f"W_ih_{s}"], np.float32)[_GATE_PERM]
        w_hh = np.asarray(inputs[f"W_hh_{s}"], np.float32)[_GATE_PERM]
        b = np.asarray(inputs[f"b_{s}"], np.float32)[_GATE_PERM]
        wt[f"wihT_{s}"] = np.ascontiguousarray(w_ih.T).astype(bf)
        wt[f"whhT_{s}"] = np.ascontiguousarray(w_hh.T).astype(bf)
        wt[f"bias_{s}"] = np.tile(b.reshape(1, G), (128, 1))
    com = {"emb": emb, "ident": np.eye(32, dtype=bf),
           "wclsT": np.ascontiguousarray(
               np.asarray(inputs["W_cls"], np.float32).T).astype(bf),
           "bcls": np.asarray(inputs["b_cls"], np.float32).reshape(TAGS, 1)}

    def chunked(a):
        return np.ascontiguousarray(a.reshape(-1).reshape(-1, 128).T)

    in_maps = [None] * NC
    for p in range(NPAIR):
        xs = x[Bl * p:Bl * (p + 1), :Tn]
        ls = np.minimum(lengths[Bl * p:Bl * (p + 1)], Tn)
        ts = np.arange(Tn)[None, :]
        rev = np.where(ts < ls[:, None], ls[:, None] - 1 - ts, ts)  # [Bl,Tn]
        base = np.arange(Bl)[:, None] * Tn + ts                      # natural
        base_rev = np.arange(Bl)[:, None] * Tn + rev                 # reversed
        m_common = {"mask": (ts < ls[:, None]).astype(np.float32)}
        m_common.update(com)

        # classifier token halves: fwd core -> seqs [0:Bl//2), bwd -> rest
        def cls_idx(b0, slotA_rev):
            tok = (np.arange(b0 * Tn, (b0 + Bl // 2) * Tn))
            bb, tt2 = tok // Tn, tok % Tn
            iA_ = bb * Tn + tt2
            iB_ = ntok + bb * Tn + rev[bb, tt2]
            if slotA_rev:
                pass
            return chunked(iA_.astype(np.int32)), chunked(iB_.astype(np.int32))

        for half, core in ((0, p), (1, p + NPAIR)):
            if half == 0:   # forward core
                m = {"xg_idx": chunked(xs),
                     "iA2": chunked(base.astype(np.int32)),
                     "iB2": chunked((ntok + base_rev).astype(np.int32)),
                     "wihT_l1": wt["wihT_f1"], "whhT_l1": wt["whhT_f1"],
                     "bias_l1": wt["bias_f1"],
                     "wihT_l2": wt["wihT_f2"], "whhT_l2": wt["whhT_f2"],
                     "bias_l2": wt["bias_f2"]}
                iA_c, iB_c = cls_idx(0, False)
            else:           # backward core
                xrev = np.take_along_axis(xs, rev, axis=1)
                m = {"xg_idx": chunked(xrev),
                     "iA2": chunked(base_rev.astype(np.int32)),
                     "iB2": chunked((ntok + base).astype(np.int32)),
                     "wihT_l1": wt["wihT_b1"], "whhT_l1": wt["whhT_b1"],
                     "bias_l1": wt["bias_b1"],
                     "wihT_l2": wt["wihT_b2"], "whhT_l2": wt["whhT_b2"],
                     "bias_l2": wt["bias_b2"]}
                iA_c, iB_c = cls_idx(Bl // 2, False)
            m["icA"], m["icB"] = iA_c, iB_c
            m.update(m_common)
            in_maps[core] = m
    return in_maps


_CACHED = {}


def kernel(**inputs) -> np.ndarray:
    if "nc" not in _CACHED:
        nc = bacc.Bacc("TRN2", target_bir_lowering=False, debug=False,
                       num_devices=NC)
        _build(nc)
        nc.compile()
        _CACHED["nc"] = nc
    nc = _CACHED["nc"]
    in_maps = _prep_inputs(inputs)
    res = run_bass_kernel_spmd(nc, in_maps, core_ids=list(range(NC)), trace=False)
    out = np.empty((B, T, TAGS), np.float32)
    for p in range(NPAIR):
        for half, core in ((0, p), (1, p + NPAIR)):
            lt = res.results[core]["logitsT"]          # [TAGS, ntok//2]
            seqs = lt.T.reshape(BL // 2, T, TAGS)
            b0 = BL * p + half * (BL // 2)
            out[b0:b0 + BL // 2] = seqs
    return out.astype(np.float32)



# revision 4
# speedup vs baseline: 1.1492x; 1.1492x over previous
"""BiLSTM tagger kernel, direction-parallel sharding over 8 NeuronCores.

Core c in 0..3 runs the FORWARD direction for sequences [32c, 32c+32);
core c+4 runs the BACKWARD direction for the same sequences (inputs
pre-reversed per sequence length on the host). Every matmul therefore has
M=32 batch rows instead of 16 — recurrent matmul cost is N-bound, so this
halves per-core PE work per step — and each core runs ONE scan per layer,
halving ScalarE/VectorE contention.

Between layers, the forward/backward halves are exchanged with a pairwise
AllGather (replica groups {c, c+4}) of the layer's scan-order output into
hpair [2*ntok, H]: slot 0 = lower rank = forward data on BOTH cores, so
the program is identical across cores; all per-core asymmetry (reversal
indices, which weights, which token half of the classifier) lives in
host-computed input tensors. Gate order i,f,o,g (sigmoid block first),
all matmuls bf16, PSUM fp32, gx injected into PSUM via identity matmuls
one step ahead.
"""

import sys

for _p in ("/opt/trn_rl_repo",):
    if _p not in sys.path:
        sys.path.append(_p)

import numpy as np
import ml_dtypes

import concourse.bass as bass
import concourse.tile as tile
from concourse import bacc, mybir
from concourse.bass import IndirectOffsetOnAxis
from concourse.bass_utils import run_bass_kernel_spmd

F32 = mybir.dt.float32
BF16 = mybir.dt.bfloat16
I32 = mybir.dt.int32
AF = mybir.ActivationFunctionType
ALU = mybir.AluOpType

B, T, V, E, H, TAGS = 128, 512, 50000, 256, 512, 64
NC = 8
NPAIR = NC // 2          # 4 sequence groups
BL = B // NPAIR          # 32 sequences per core (one direction each)
G = 4 * H
GROUPS = [[c, c + NPAIR] for c in range(NPAIR)]

# gate order i,f,o,g: quadrant q of the col-tiled psum holds gate q, so
# sigmoid covers partitions 0:96 in ONE activation op and tanh(g) is the
# 32-partition op at base 96
_GATE_PERM = np.concatenate([
    np.arange(0, H), np.arange(H, 2 * H), np.arange(3 * H, 4 * H),
    np.arange(2 * H, 3 * H)])


def _build(nc, Tn=T, Bl=BL, TC=2, RC=4):
    ntok = Bl * Tn            # 16384 per core
    nchunk = ntok // 128      # 128
    ncls = ntok // 2 // 128   # 64 classifier chunks (half the pair's tokens)
    KE = E // 128
    KH2 = 2 * H // 128
    KH = H // 128

    # ---- dram I/O (per-core data resolves fwd/bwd asymmetry) ----
    emb = nc.dram_tensor("emb", [V, E], F32, kind="ExternalInput")
    xg_idx = nc.dram_tensor("xg_idx", [128, nchunk], I32, kind="ExternalInput")
    iA2 = nc.dram_tensor("iA2", [128, nchunk], I32, kind="ExternalInput")
    iB2 = nc.dram_tensor("iB2", [128, nchunk], I32, kind="ExternalInput")
    icA = nc.dram_tensor("icA", [128, ncls], I32, kind="ExternalInput")
    icB = nc.dram_tensor("icB", [128, ncls], I32, kind="ExternalInput")
    mask = nc.dram_tensor("mask", [Bl, Tn], F32, kind="ExternalInput")
    ident = nc.dram_tensor("ident", [32, 32], BF16, kind="ExternalInput")

    wih, whh, biasd = {}, {}, {}
    for l, din in (("l1", E), ("l2", 2 * H)):
        wih[l] = nc.dram_tensor(f"wihT_{l}", [din, G], BF16, kind="ExternalInput")
        whh[l] = nc.dram_tensor(f"whhT_{l}", [H, G], BF16, kind="ExternalInput")
        biasd[l] = nc.dram_tensor(f"bias_{l}", [128, G], F32, kind="ExternalInput")
    wcls = nc.dram_tensor("wclsT", [2 * H, TAGS], BF16, kind="ExternalInput")
    bcls = nc.dram_tensor("bcls", [TAGS, 1], F32, kind="ExternalInput")

    gx = {l: nc.dram_tensor(f"gx_{l}", [ntok, G], BF16) for l in ("l1", "l2")}
    hloc = {l: nc.dram_tensor(f"hloc_{l}", [ntok, H], BF16) for l in ("l1", "l2")}
    hpair = {l: nc.dram_tensor(f"hpair_{l}", [2 * ntok, H], BF16)
             for l in ("l1", "l2")}
    logitsT = nc.dram_tensor("logitsT", [TAGS, ntok // 2], F32,
                             kind="ExternalOutput")

    with tile.TileContext(nc) as tc:
        with tc.tile_pool(name="const", bufs=1) as cpool:
            def load_const(nm, shape, dt, src_ap):
                t = cpool.tile(shape, dt, name=nm, tag=nm)
                nc.gpsimd.dma_start(t[:], src_ap)
                return t

            xg_sb = load_const("xg_sb", [128, nchunk], I32, xg_idx[:])
            iA2_sb = load_const("iA2_sb", [128, nchunk], I32, iA2[:])
            iB2_sb = load_const("iB2_sb", [128, nchunk], I32, iB2[:])
            icA_sb = load_const("icA_sb", [128, ncls], I32, icA[:])
            icB_sb = load_const("icB_sb", [128, ncls], I32, icB[:])
            mask_sb = load_const("mask_sb", [Bl, Tn], F32, mask[:])
            id_sb = load_const("id_sb", [32, 32], BF16, ident[:])
            bcls_sb = load_const("bcls_sb", [TAGS, 1], F32, bcls[:])
            bias_sb = {l: load_const(f"bias_sb_{l}", [128, G], F32, biasd[l][:])
                       for l in ("l1", "l2")}
            wcls_sb = cpool.tile([128, KH2, TAGS], BF16, name="wcls_sb")
            for k in range(KH2):
                nc.gpsimd.dma_start(wcls_sb[:, k, :], wcls[128 * k:128 * (k + 1), :])

            # layer-1 proj (emb gather) + scan + exchange
            _proj(nc, tc, nchunk, KE, wih["l1"], bias_sb["l1"], gx["l1"],
                  emb, xg_sb, None, None, is_emb=True)
            _scan(nc, tc, Tn, Bl, TC, RC, KH, whh["l1"], gx["l1"], hloc["l1"],
                  mask_sb, id_sb)
            nc.gpsimd.collective_compute(
                "AllGather", ALU.bypass, GROUPS,
                ins=[hloc["l1"][:]], outs=[hpair["l1"][:]])
            # layer-2 proj (gathers from hpair) + scan + exchange
            _proj(nc, tc, nchunk, KH2, wih["l2"], bias_sb["l2"], gx["l2"],
                  hpair["l1"], None, iA2_sb, iB2_sb, is_emb=False)
            _scan(nc, tc, Tn, Bl, TC, RC, KH, whh["l2"], gx["l2"], hloc["l2"],
                  mask_sb, id_sb)
            nc.gpsimd.collective_compute(
                "AllGather", ALU.bypass, GROUPS,
                ins=[hloc["l2"][:]], outs=[hpair["l2"][:]])

            # classifier over this core's half of the pair's tokens
            with tc.tile_pool(name="cls", bufs=3) as gp, \
                 tc.tile_pool(name="clsT", bufs=3) as gtp, \
                 tc.tile_pool(name="clsps", bufs=4, space="PSUM") as pp, \
                 tc.tile_pool(name="clso", bufs=3) as op:
                for c in range(ncls):
                    o2 = gp.tile([128, 2 * H], BF16, tag="in")
                    nc.gpsimd.indirect_dma_start(
                        out=o2[:, 0:H], out_offset=None, in_=hpair["l2"][:],
                        in_offset=IndirectOffsetOnAxis(ap=icA_sb[:, c:c + 1], axis=0))
                    nc.gpsimd.indirect_dma_start(
                        out=o2[:, H:2 * H], out_offset=None, in_=hpair["l2"][:],
                        in_offset=IndirectOffsetOnAxis(ap=icB_sb[:, c:c + 1], axis=0))
                    o2T = gtp.tile([128, KH2, 128], BF16, tag="inT")
                    for k in range(KH2):
                        nc.sync.dma_start_transpose(
                            o2T[:, k, :], o2[:, 128 * k:128 * (k + 1)])
                    ps = pp.tile([TAGS, 128], F32, name="clsps_t")
                    for k in range(KH2):
                        nc.tensor.matmul(ps[:], wcls_sb[:, k, :], o2T[:, k, :],
                                         start=(k == 0), stop=(k == KH2 - 1))
                    lg = op.tile([TAGS, 128], F32, tag="lg")
                    nc.scalar.activation(lg[:], ps[:], AF.Identity,
                                         bias=bcls_sb[:, 0:1])
                    nc.gpsimd.dma_start(logitsT[:, 128 * c:128 * (c + 1)], lg[:])
    return nc


def _proj(nc, tc, nchunk, KD, wih_d, bias_t, gx_d, src, emb_idx, iA, iB, is_emb):
    """gx = input @ W_ih^T + b in scan-time order. Layer 1: fp32 emb row
    gather + cast. Layer 2: two bf16 row gathers from hpair."""
    D = KD * 128
    G_ = G
    with tc.tile_pool(name="pw", bufs=1) as wpool, \
         tc.tile_pool(name="pg", bufs=3) as gpool, \
         tc.tile_pool(name="pgT", bufs=3) as tpool, \
         tc.tile_pool(name="pps", bufs=4, space="PSUM") as ppool, \
         tc.tile_pool(name="pout", bufs=3) as opool:
        wsb = wpool.tile([128, KD, G_], BF16, tag="w", name="wih_sb")
        for k in range(KD):
            nc.gpsimd.dma_start(wsb[:, k, :], wih_d[128 * k:128 * (k + 1), :])
        for c in range(nchunk):
            if is_emb:
                e32 = gpool.tile([128, D], F32, tag="e32")
                nc.gpsimd.indirect_dma_start(
                    out=e32[:], out_offset=None, in_=src[:],
                    in_offset=IndirectOffsetOnAxis(ap=emb_idx[:, c:c + 1], axis=0))
                xin = gpool.tile([128, D], BF16, tag="e16")
                nc.vector.tensor_copy(xin[:], e32[:])
            else:
                xin = gpool.tile([128, D], BF16, tag="e16")
                nc.gpsimd.indirect_dma_start(
                    out=xin[:, 0:H], out_offset=None, in_=src[:],
                    in_offset=IndirectOffsetOnAxis(ap=iA[:, c:c + 1], axis=0))
                nc.gpsimd.indirect_dma_start(
                    out=xin[:, H:2 * H], out_offset=None, in_=src[:],
                    in_offset=IndirectOffsetOnAxis(ap=iB[:, c:c + 1], axis=0))
            xT = tpool.tile([128, KD, 128], BF16, tag="xT")
            for k in range(KD):
                nc.sync.dma_start_transpose(xT[:, k, :], xin[:, 128 * k:128 * (k + 1)])
            gout = opool.tile([128, G_], BF16, tag="gout")
            for n in range(G_ // 512):
                ps = ppool.tile([128, 512], F32, name="pps")
                for k in range(KD):
                    nc.tensor.matmul(
                        ps[:], xT[:, k, :], wsb[:, k, 512 * n:512 * (n + 1)],
                        start=(k == 0), stop=(k == KD - 1))
                nc.vector.tensor_tensor(
                    out=gout[:, 512 * n:512 * (n + 1)], in0=ps[:],
                    in1=bias_t[:, 512 * n:512 * (n + 1)], op=ALU.add)
            nc.gpsimd.dma_start(gx_d[128 * c:128 * (c + 1), :], gout[:])


def _scan(nc, tc, Tn, Bl, TC, RC, KH, whh_d, gx_d, hout_d, mask_sb, id_sb):
    """Single-direction scan, M=32. Col-tiled quadrant psum layout: the 16
    recurrent matmuls run as 4 rounds of 4 CONCURRENT col-tiled matmuls
    (tile_position=(0,32q)); psum quadrant q = gate q (order i,f,o,g), so
    sigmoid(i,f,o) is ONE [96,512] activation and tanh(g) one [32,512] op.
    f/o/g are realigned to base partition 0 with cheap cross-quadrant DVE
    copies (hidden under ACT ops); all elementwise runs at base 0."""
    gxv = gx_d.ap().rearrange("(b t) d -> b t d", b=Bl)
    houtv = hout_d.ap().rearrange("(b t) d -> b t d", b=Bl)
    with tc.tile_pool(name="sw", bufs=1) as wpool, \
         tc.tile_pool(name="sgx", bufs=4) as gxpool, \
         tc.tile_pool(name="sst", bufs=1) as stpool, \
         tc.tile_pool(name="sps", bufs=2, space="PSUM") as pspool, \
         tc.tile_pool(name="stps", bufs=2, space="PSUM") as tpspool, \
         tc.tile_pool(name="swk", bufs=3) as wkpool, \
         tc.tile_pool(name="shT", bufs=3) as htpool, \
         tc.tile_pool(name="srng", bufs=3) as rpool:
        wsb = wpool.tile([128, KH, G], BF16, tag="whh", name="whh_sb")
        for k in range(KH):
            nc.gpsimd.dma_start(wsb[:, k, :], whh_d[128 * k:128 * (k + 1), :])
        hT = [htpool.tile([128, KH * Bl], BF16, tag="hT", name="hT0")]
        nc.vector.memset(hT[0][:], 0.0)
        c_st = stpool.tile([Bl, H], F32, tag="c", name="c_st")
        nc.vector.memset(c_st[:], 0.0)
        gxc = {}
        gps = [None]
        ring = [None]
        nwin = (Tn + TC - 1) // TC

        def load_gx(w):
            tl = gxpool.tile([Bl, TC, G], BF16, tag="gx", name="gxc")
            nc.gpsimd.dma_start(tl[:], gxv[:, w * TC:(w + 1) * TC, :])
            gxc[w] = tl
            gxc.pop(w - 3, None)

        def inject(tt):
            gps[0] = pspool.tile([128, H], F32, tag="ps", name="gps")
            gxt = gxc[tt // TC]
            for q in range(4):
                nc.tensor.matmul(
                    gps[0][32 * q:32 * (q + 1), :], id_sb[:],
                    gxt[:, tt % TC, 512 * q:512 * (q + 1)],
                    start=True, stop=False, tile_position=(0, 32 * q),
                    skip_group_check=True)

        load_gx(0)
        if nwin > 1:
            load_gx(1)
        inject(0)
        for t in range(Tn):
            gc = gps[0]
            # 4 rounds x 4 concurrent col-tiled matmuls
            for k in range(KH):
                for q in range(4):
                    nc.tensor.matmul(
                        gc[32 * q:32 * (q + 1), :],
                        hT[0][:, Bl * k:Bl * (k + 1)],
                        wsb[:, k, 512 * q:512 * (q + 1)],
                        start=False, stop=(k == KH - 1),
                        tile_position=(0, 32 * q), skip_group_check=True)
            gact = wkpool.tile([128, H], BF16, tag="gact", name="gact")
            # one op for sigmoid over i,f,o (quadrants 0..2)
            nc.scalar.activation(gact[0:96, :], gc[0:96, :], AF.Sigmoid)
            # f realigned to base 0 while tanh(g) runs on ScalarE
            gf0 = wkpool.tile([Bl, H], BF16, tag="gf0", name="gf0")
            nc.vector.tensor_copy(gf0[:], gact[32:64, :])
            # tanh(g) written cross-partition directly to base 0
            gg0 = wkpool.tile([Bl, H], BF16, tag="gg0", name="gg0")
            nc.scalar.activation(gg0[:], gc[96:128, :], AF.Tanh)
            t1 = wkpool.tile([Bl, H], F32, tag="t1", name="t1")
            nc.vector.tensor_tensor(out=t1[:], in0=gf0[:], in1=c_st[:],
                                    op=ALU.mult)
            t2 = wkpool.tile([Bl, H], BF16, tag="t2", name="t2")
            nc.vector.tensor_tensor(out=t2[:], in0=gact[0:32, :], in1=gg0[:],
                                    op=ALU.mult)
            nc.vector.tensor_tensor(out=c_st[:], in0=t1[:], in1=t2[:], op=ALU.add)
            tch = wkpool.tile([Bl, H], BF16, tag="tch", name="tch")
            nc.scalar.activation(tch[:], c_st[:], AF.Tanh)
            go0 = wkpool.tile([Bl, H], BF16, tag="go0", name="go0")
            nc.vector.tensor_copy(go0[:], gact[64:96, :])
            h16 = wkpool.tile([Bl, H], BF16, tag="h16", name="h16")
            nc.vector.tensor_tensor(out=h16[:], in0=go0[:], in1=tch[:],
                                    op=ALU.mult)
            if t % RC == 0:
                ring[0] = rpool.tile([Bl, RC, H], BF16, tag="ring", name="ring")
            hT_ps = tpspool.tile([128, KH * Bl], F32, tag="tps", name="hT_ps")
            if t + 1 < Tn:
                if (t + 1) % TC == 0 and (t + 1) // TC + 1 < nwin:
                    load_gx((t + 1) // TC + 1)
                inject(t + 1)
            hTn = htpool.tile([128, KH * Bl], BF16, tag="hT", name="hTn")
            for k in range(KH):
                nc.tensor.matmul(hT_ps[:, Bl * k:Bl * (k + 1)],
                                 h16[:, 128 * k:128 * (k + 1)], id_sb[:],
                                 start=True, stop=True, skip_group_check=True)
            # single copy of all 4 transposed chunks back to SBUF
            nc.scalar.activation(hTn[:], hT_ps[:], AF.Copy)
            hT[0] = hTn
            nc.vector.tensor_scalar_mul(ring[0][:, t % RC, :], h16[:],
                                        mask_sb[:, t:t + 1])
            if (t + 1) % RC == 0:
                t0r = t + 1 - RC
                nc.gpsimd.dma_start(houtv[:, t0r:t0r + RC, :], ring[0][:, :, :])


def _prep_inputs(inputs, Tn=T, Bl=BL):
    x = np.asarray(inputs["x"]).astype(np.int32)
    lengths = np.asarray(inputs["lengths"]).astype(np.int32)
    emb = np.asarray(inputs["emb"], dtype=np.float32)
    ntok = Bl * Tn
    bf = ml_dtypes.bfloat16

    wt = {}
    for s in ("f1", "b1", "f2", "b2"):
        w_ih = np.asarray(inputs[f"W_ih_{s}"], np.float32)[_GATE_PERM]
        w_hh = np.asarray(inputs[f"W_hh_{s}"], np.float32)[_GATE_PERM]
        b = np.asarray(inputs[f"b_{s}"], np.float32)[_GATE_PERM]
        wt[f"wihT_{s}"] = np.ascontiguousarray(w_ih.T).astype(bf)
        wt[f"whhT_{s}"] = np.ascontiguousarray(w_hh.T).astype(bf)
        wt[f"bias_{s}"] = np.tile(b.reshape(1, G), (128, 1))
    com = {"emb": emb, "ident": np.eye(32, dtype=bf),
           "wclsT": np.ascontiguousarray(
               np.asarray(inputs["W_cls"], np.float32).T).astype(bf),
           "bcls": np.asarray(inputs["b_cls"], np.float32).reshape(TAGS, 1)}

    def chunked(a):
        return np.ascontiguousarray(a.reshape(-1).reshape(-1, 128).T)

    in_maps = [None] * NC
    for p in range(NPAIR):
        xs = x[Bl * p:Bl * (p + 1), :Tn]
        ls = np.minimum(lengths[Bl * p:Bl * (p + 1)], Tn)
        ts = np.arange(Tn)[None, :]
        rev = np.where(ts < ls[:, None], ls[:, None] - 1 - ts, ts)  # [Bl,Tn]
        base = np.arange(Bl)[:, None] * Tn + ts                      # natural
        base_rev = np.arange(Bl)[:, None] * Tn + rev                 # reversed
        m_common = {"mask": (ts < ls[:, None]).astype(np.float32)}
        m_common.update(com)

        # classifier token halves: fwd core -> seqs [0:Bl//2), bwd -> rest
        def cls_idx(b0, slotA_rev):
            tok = (np.arange(b0 * Tn, (b0 + Bl // 2) * Tn))
            bb, tt2 = tok // Tn, tok % Tn
            iA_ = bb * Tn + tt2
            iB_ = ntok + bb * Tn + rev[bb, tt2]
            if slotA_rev:
                pass
            return chunked(iA_.astype(np.int32)), chunked(iB_.astype(np.int32))

        for half, core in ((0, p), (1, p + NPAIR)):
            if half == 0:   # forward core
                m = {"xg_idx": chunked(xs),
                     "iA2": chunked(base.astype(np.int32)),
                     "iB2": chunked((ntok + base_rev).astype(np.int32)),
                     "wihT_l1": wt["wihT_f1"], "whhT_l1": wt["whhT_f1"],
                     "bias_l1": wt["bias_f1"],
                     "wihT_l2": wt["wihT_f2"], "whhT_l2": wt["whhT_f2"],
                     "bias_l2": wt["bias_f2"]}
                iA_c, iB_c = cls_idx(0, False)
            else:           # backward core
                xrev = np.take_along_axis(xs, rev, axis=1)
                m = {"xg_idx": chunked(xrev),
                     "iA2": chunked(base_rev.astype(np.int32)),
                     "iB2": chunked((ntok + base).astype(np.int32)),
                     "wihT_l1": wt["wihT_b1"], "whhT_l1": wt["whhT_b1"],
                     "bias_l1": wt["bias_b1"],
                     "wihT_l2": wt["wihT_b2"], "whhT_l2": wt["whhT_b2"],
                     "bias_l2": wt["bias_b2"]}
                iA_c, iB_c = cls_idx(Bl // 2, False)
            m["icA"], m["icB"] = iA_c, iB_c
            m.update(m_common)
            in_maps[core] = m
    return in_maps


_CACHED = {}


def kernel(**inputs) -> np.ndarray:
    if "nc" not in _CACHED:
        nc = bacc.Bacc("TRN2", target_bir_lowering=False, debug=False,
                       num_devices=NC)
        _build(nc)
        nc.compile()
        _CACHED["nc"] = nc
    nc = _CACHED["nc"]
    in_maps = _prep_inputs(inputs)
    res = run_bass_kernel_spmd(nc, in_maps, core_ids=list(range(NC)), trace=False)
    out = np.empty((B, T, TAGS), np.float32)
    for p in range(NPAIR):
        for half, core in ((0, p), (1, p + NPAIR)):
            lt = res.results[core]["logitsT"]          # [TAGS, ntok//2]
            seqs = lt.T.reshape(BL // 2, T, TAGS)
            b0 = BL * p + half * (BL // 2)
            out[b0:b0 + BL // 2] = seqs
    return out.astype(np.float32)



# revision 12
# speedup vs baseline: 1.1703x; 1.0183x over previous
"""BiLSTM tagger kernel, direction-parallel over 8 NeuronCores, v4.

Core c in 0..3 runs the FORWARD direction for sequences [32c, 32c+32);
core c+4 runs the BACKWARD direction for the same sequences. The backward
scan runs over GLOBALLY reversed time (host flips x along t); per-sequence
ragged reversal is handled by masking the i/f/o gates each step (mask=0
keeps h=c=0 through the pad region), which reproduces pad-packed semantics
exactly with no per-sequence index tensors.

Tokens are TIME-MAJOR (tok = t*32 + b) and all inter-phase tensors live in
TRANSPOSED layout hT[128, Tn*128] (col block t = 4 H-chunks x 32 batch),
written directly from the scan's per-step PE transpose. Backward cores
write their hT at column T-1-t, so every stored tensor is in forward time
order and the layer-2 projection + classifier read PLAIN contiguous
blocks: no indirect gathers and no DMA transposes anywhere. The backward
core's own layer-2 projection reads forward-time blocks as-is (inner
4-step order reversed) and its scan un-permutes by indexing the gx window
with 3-(t%4).

Scan: col-tiled quadrant psum layout - the 16 recurrent matmuls run as 4
rounds of 4 CONCURRENT col-tiled matmuls (tile_position=(0,32q)); psum
quadrant q = gate q (order i,f,o,g), sigmoid(i,f,o) is ONE [96,512] op.
All matmuls bf16, psum fp32.
"""

import sys

for _p in ("/opt/trn_rl_repo",):
    if _p not in sys.path:
        sys.path.append(_p)

import numpy as np
import ml_dtypes

import concourse.bass as bass
import concourse.tile as tile
from concourse import bacc, mybir
from concourse.bass import IndirectOffsetOnAxis
from concourse.bass_utils import run_bass_kernel_spmd

F32 = mybir.dt.float32
BF16 = mybir.dt.bfloat16
I32 = mybir.dt.int32
AF = mybir.ActivationFunctionType
ALU = mybir.AluOpType

B, T, V, E, H, TAGS = 128, 512, 50000, 256, 512, 64
NC = 8
NPAIR = NC // 2
BL = B // NPAIR          # 32 sequences per core (one direction each)
G = 4 * H
GROUPS = [[c, c + NPAIR] for c in range(NPAIR)]
NTOK = BL * T            # 16384 tokens per core, time-major: tok = t*32+b

# gate order i,f,o,g: psum quadrant q holds gate q, sigmoid covers 0:96
_GATE_PERM = np.concatenate([
    np.arange(0, H), np.arange(H, 2 * H), np.arange(3 * H, 4 * H),
    np.arange(2 * H, 3 * H)])


def _build(nc, Tn=T, Bl=BL):
    nchunk = NTOK // 128     # 128 token chunks (4 steps each)
    ncls = nchunk // 2       # 64 classifier chunks per core
    KE = E // 128            # 2
    KH = H // 128            # 4
    KH2 = 2 * H // 128       # 8

    emb = nc.dram_tensor("emb", [V, E], F32, kind="ExternalInput")
    xg_idx = nc.dram_tensor("xg_idx", [128, nchunk], I32, kind="ExternalInput")
    mask = nc.dram_tensor("mask", [Bl, Tn], F32, kind="ExternalInput")
    ident = nc.dram_tensor("ident", [32, 32], BF16, kind="ExternalInput")
    id128 = nc.dram_tensor("id128", [128, 128], BF16, kind="ExternalInput")
    flagF = nc.dram_tensor("flagF", [128, 1], F32, kind="ExternalInput")
    flagB = nc.dram_tensor("flagB", [128, 1], F32, kind="ExternalInput")

    wih, whh, biasd = {}, {}, {}
    for l, din in (("l1", E), ("l2", 2 * H)):
        wih[l] = nc.dram_tensor(f"wihT_{l}", [din, G], BF16, kind="ExternalInput")
        whh[l] = nc.dram_tensor(f"whhT_{l}", [H, G], BF16, kind="ExternalInput")
        biasd[l] = nc.dram_tensor(f"bias_{l}", [128, G], F32, kind="ExternalInput")
    wcls = nc.dram_tensor("wclsT", [2 * H, TAGS], BF16, kind="ExternalInput")
    bcls = nc.dram_tensor("bcls", [TAGS, 1], F32, kind="ExternalInput")

    gx = {l: nc.dram_tensor(f"gx_{l}", [NTOK, G], BF16) for l in ("l1", "l2")}
    hT = {l: nc.dram_tensor(f"hT_{l}", [128, Tn * 128], BF16)
          for l in ("l1", "l2")}
    hTp = {l: nc.dram_tensor(f"hTp_{l}", [2 * 128, Tn * 128], BF16)
           for l in ("l1", "l2")}
    logitsT = nc.dram_tensor("logitsT", [TAGS, NTOK // 2], F32,
                             kind="ExternalOutput")

    # Every core writes its hT in OWN-scan order (uniform). In the
    # exchanged buffer slot0 = fwd core's hT (columns = fwd time) and
    # slot1 = bwd core's (columns = bwd scan time = T-1-fwd). A core
    # reading x2 at its own scan step t needs the OWN slot natural and
    # the CROSS slot time-reversed; which slot is which depends on the
    # core, so both variants are loaded (cheap contiguous DMA) and
    # selected with a host 0/1 flag on the DVE (SPMD-uniform program).

    with tile.TileContext(nc) as tc:
        with tc.tile_pool(name="const", bufs=1) as cpool:
            def load_const(nm, shape, dt, src_ap):
                t = cpool.tile(shape, dt, name=nm, tag=nm)
                nc.gpsimd.dma_start(t[:], src_ap)
                return t

            xg_sb = load_const("xg_sb", [128, nchunk], I32, xg_idx[:])
            mask_sb = load_const("mask_sb", [Bl, Tn], F32, mask[:])
            id_sb = load_const("id_sb", [32, 32], BF16, ident[:])
            id128_sb = load_const("id128_sb", [128, 128], BF16, id128[:])
            bcls_sb = load_const("bcls_sb", [TAGS, 1], F32, bcls[:])
            fF_sb = load_const("fF_sb", [128, 1], F32, flagF[:])
            fB_sb = load_const("fB_sb", [128, 1], F32, flagB[:])
            bias_sb = {l: load_const(f"bias_sb_{l}", [128, G], F32, biasd[l][:])
                       for l in ("l1", "l2")}

            _proj1(nc, tc, nchunk, KE, wih["l1"], bias_sb["l1"], gx["l1"],
                   emb, xg_sb, id128_sb)
            _scan(nc, tc, Tn, Bl, KH, whh["l1"], gx["l1"], hT["l1"],
                  mask_sb, id_sb)
            nc.gpsimd.collective_compute(
                "AllGather", ALU.bypass, GROUPS,
                ins=[hT["l1"][:]], outs=[hTp["l1"][:]])
            _proj2(nc, tc, nchunk, Tn, wih["l2"], bias_sb["l2"], gx["l2"],
                   hTp["l1"], fF_sb, fB_sb)
            _scan(nc, tc, Tn, Bl, KH, whh["l2"], gx["l2"], hT["l2"],
                  mask_sb, id_sb)
            nc.gpsimd.collective_compute(
                "AllGather", ALU.bypass, GROUPS,
                ins=[hT["l2"][:]], outs=[hTp["l2"][:]])
            _classifier(nc, tc, ncls, Tn, wcls, bcls_sb, hTp["l2"], logitsT,
                        KH2, fF_sb, fB_sb)
    return nc


def _proj1(nc, tc, nchunk, KE, wih_d, bias_t, gx_d, emb, xg_sb, id128_sb):
    """gx1 = emb[x] @ W_ih1^T + b, time-major chunks of 128 tokens.
    Embedding rows gathered (fp32), cast, transposed on the PE."""
    with tc.tile_pool(name="pw", bufs=1) as wpool, \
         tc.tile_pool(name="pg", bufs=3) as gpool, \
         tc.tile_pool(name="pxps", bufs=2, space="PSUM") as xpspool, \
         tc.tile_pool(name="pps", bufs=4, space="PSUM") as ppool, \
         tc.tile_pool(name="pout", bufs=3) as opool:
        wsb = wpool.tile([128, KE, G], BF16, tag="w", name="wih1_sb")
        for k in range(KE):
            nc.gpsimd.dma_start(wsb[:, k, :], wih_d[128 * k:128 * (k + 1), :])
        for s in range(nchunk):
            e32 = gpool.tile([128, E], F32, tag="e32")
            nc.gpsimd.indirect_dma_start(
                out=e32[:], out_offset=None, in_=emb[:],
                in_offset=IndirectOffsetOnAxis(ap=xg_sb[:, s:s + 1], axis=0))
            e16 = gpool.tile([128, E], BF16, tag="e16")
            nc.vector.tensor_copy(e16[:], e32[:])
            xps = xpspool.tile([128, E], F32, tag="xps", name="xps")
            for kk in range(KE):
                nc.tensor.matmul(xps[:, 128 * kk:128 * (kk + 1)],
                                 e16[:, 128 * kk:128 * (kk + 1)], id128_sb[:],
                                 start=True, stop=True, skip_group_check=True)
            xT = gpool.tile([128, E], BF16, tag="xT")
            nc.scalar.activation(xT[:], xps[:], AF.Copy)
            gout = opool.tile([128, G], BF16, tag="gout")
            for n in range(4):
                ps = ppool.tile([128, 512], F32, tag="ps", name="pps")
                for kk in range(KE):
                    nc.tensor.matmul(
                        ps[:], xT[:, 128 * kk:128 * (kk + 1)],
                        wsb[:, kk, 512 * n:512 * (n + 1)],
                        start=(kk == 0), stop=(kk == KE - 1),
                        skip_group_check=True)
                nc.vector.tensor_tensor(
                    out=gout[:, 512 * n:512 * (n + 1)], in0=ps[:],
                    in1=bias_t[:, 512 * n:512 * (n + 1)], op=ALU.add)
            nc.gpsimd.dma_start(gx_d[128 * s:128 * (s + 1), :], gout[:])


def _load_x2(nc, xpool, hv, s, Tn, fF, fB, tag):
    """Load x2^T [128, 8, 4, 32] for chunk s (this core's scan steps
    4s..4s+4): slot d chunks k at rows 128d.. Both the natural and the
    time-reversed variant of each slot are loaded (contiguous DMA) and
    blended with the core's 0/1 flags: own slot natural, cross reversed.
    hv dims: [d, p, k, t, c32] (k before t so AP dim order matches dst)."""
    xn = xpool.tile([128, 8, 4, 32], BF16, tag=tag + "n")
    xr = xpool.tile([128, 8, 4, 32], BF16, tag=tag + "r")
    hi = Tn - 1 - 4 * s
    rsl = slice(hi, None, -1) if hi - 4 < 0 else slice(hi, hi - 4, -1)
    for d in range(2):
        for k in range(4):
            nc.gpsimd.dma_start(xn[:, 4 * d + k, :, :],
                                hv[d, :, k, 4 * s:4 * s + 4, :])
            nc.gpsimd.dma_start(xr[:, 4 * d + k, :, :],
                                hv[d, :, k, rsl, :])
    xT = xpool.tile([128, 8, 4, 32], BF16, tag=tag)
    a = xpool.tile([128, 4, 4, 32], BF16, tag=tag + "a")
    b = xpool.tile([128, 4, 4, 32], BF16, tag=tag + "b")
    # slot0 (fwd dir): natural on fwd cores, reversed on bwd cores
    nc.vector.tensor_scalar_mul(a[:], xn[:, 0:4, :, :], fF[:, 0:1])
    nc.vector.tensor_scalar_mul(b[:], xr[:, 0:4, :, :], fB[:, 0:1])
    nc.vector.tensor_tensor(out=xT[:, 0:4, :, :], in0=a[:], in1=b[:],
                            op=ALU.add)
    # slot1 (bwd dir): reversed on fwd cores, natural on bwd cores
    nc.vector.tensor_scalar_mul(a[:], xn[:, 4:8, :, :], fB[:, 0:1])
    nc.vector.tensor_scalar_mul(b[:], xr[:, 4:8, :, :], fF[:, 0:1])
    nc.vector.tensor_tensor(out=xT[:, 4:8, :, :], in0=a[:], in1=b[:],
                            op=ALU.add)
    return xT


def _proj2(nc, tc, nchunk, Tn, wih_d, bias_t, gx_d, hTp_d, fF, fB):
    """gx2 = [out_f | out_b] @ W_ih2^T + b in this core's own scan order."""
    hv = hTp_d.ap().rearrange("(d p) (t k c) -> d p k t c", d=2, k=4, c=32)
    KD = 8
    with tc.tile_pool(name="qw", bufs=1) as wpool, \
         tc.tile_pool(name="qx", bufs=3) as xpool, \
         tc.tile_pool(name="qps", bufs=4, space="PSUM") as ppool, \
         tc.tile_pool(name="qout", bufs=3) as opool:
        wsb = wpool.tile([128, KD, G], BF16, tag="w", name="wih2_sb")
        for k in range(KD):
            nc.gpsimd.dma_start(wsb[:, k, :], wih_d[128 * k:128 * (k + 1), :])
        for s in range(nchunk):
            xT = _load_x2(nc, xpool, hv, s, Tn, fF, fB, "xT")
            gout = opool.tile([128, G], BF16, tag="gout")
            for n in range(4):
                ps = ppool.tile([128, 512], F32, tag="ps", name="qpps")
                for kk in range(KD):
                    nc.tensor.matmul(
                        ps[:], xT[:, kk, :, :],
                        wsb[:, kk, 512 * n:512 * (n + 1)],
                        start=(kk == 0), stop=(kk == KD - 1),
                        skip_group_check=True)
                nc.vector.tensor_tensor(
                    out=gout[:, 512 * n:512 * (n + 1)], in0=ps[:],
                    in1=bias_t[:, 512 * n:512 * (n + 1)], op=ALU.add)
            nc.gpsimd.dma_start(gx_d[128 * s:128 * (s + 1), :], gout[:])


def _classifier(nc, tc, ncls, Tn, wcls_d, bcls_sb, hTp_d, logitsT, KH2,
                fF, fB):
    """logits for this core's half of the pair's tokens: chunks s=0..63 of
    its OWN scan time (host un-reverses bwd cores)."""
    hv = hTp_d.ap().rearrange("(d p) (t k c) -> d p k t c", d=2, k=4, c=32)
    with tc.tile_pool(name="cw", bufs=1) as wpool, \
         tc.tile_pool(name="cx", bufs=3) as xpool, \
         tc.tile_pool(name="cps", bufs=4, space="PSUM") as ppool, \
         tc.tile_pool(name="cout", bufs=3) as opool:
        wsb = wpool.tile([128, KH2, TAGS], BF16, tag="w", name="wcls_sb")
        for k in range(KH2):
            nc.gpsimd.dma_start(wsb[:, k, :], wcls_d[128 * k:128 * (k + 1), :])
        for s in range(ncls):
            o2T = _load_x2(nc, xpool, hv, s, Tn, fF, fB, "o2T")
            ps = ppool.tile([TAGS, 128], F32, tag="ps", name="cpps")
            for kk in range(KH2):
                nc.tensor.matmul(ps[:], wsb[:, kk, :], o2T[:, kk, :, :],
                                 start=(kk == 0), stop=(kk == KH2 - 1),
                                 skip_group_check=True)
            lg = opool.tile([TAGS, 128], F32, tag="lg")
            nc.scalar.activation(lg[:], ps[:], AF.Identity,
                                 bias=bcls_sb[:, 0:1])
            nc.gpsimd.dma_start(logitsT[:, 128 * s:128 * (s + 1)], lg[:])


def _scan(nc, tc, Tn, Bl, KH, whh_d, gx_d, hTout_d, mask_sb, id_sb):
    """Single-direction scan, M=32, col-tiled quadrant psum layout.
    Gate masks (i,f,o multiplied by mask[:,t]) implement pad-packed
    semantics; the transposed state hTn is DMA'd per step straight into
    hTout (this core's scan order)."""
    TC = 4
    gxv = gx_d.ap().rearrange("(t b) d -> b t d", b=Bl)
    hTv = hTout_d.ap().rearrange("p (t c) -> p t c", c=128)
    with tc.tile_pool(name="sw", bufs=1) as wpool, \
         tc.tile_pool(name="sgx", bufs=3) as gxpool, \
         tc.tile_pool(name="sst", bufs=1) as stpool, \
         tc.tile_pool(name="sps", bufs=2, space="PSUM") as pspool, \
         tc.tile_pool(name="stps", bufs=2, space="PSUM") as tpspool, \
         tc.tile_pool(name="swk", bufs=3) as wkpool, \
         tc.tile_pool(name="shT", bufs=3) as htpool:
        wsb = wpool.tile([128, KH, G], BF16, tag="whh", name="whh_sb")
        for k in range(KH):
            nc.gpsimd.dma_start(wsb[:, k, :], whh_d[128 * k:128 * (k + 1), :])
        hT = [htpool.tile([128, KH * Bl], BF16, tag="hT", name="hT0")]
        nc.vector.memset(hT[0][:], 0.0)
        c_st = stpool.tile([Bl, H], F32, tag="c", name="c_st")
        nc.vector.memset(c_st[:], 0.0)
        gxc = {}
        gps = [None]
        nwin = (Tn + TC - 1) // TC

        def load_gx(w):
            tl = gxpool.tile([Bl, TC, G], BF16, tag="gx", name="gxc")
            nc.gpsimd.dma_start(tl[:], gxv[:, w * TC:(w + 1) * TC, :])
            gxc[w] = tl
            gxc.pop(w - 2, None)

        def inject(tt):
            gps[0] = pspool.tile([128, H], F32, tag="ps", name="gps")
            gxt = gxc[tt // TC]
            j = tt % TC
            for q in range(4):
                nc.tensor.matmul(
                    gps[0][32 * q:32 * (q + 1), :], id_sb[:],
                    gxt[:, j, 512 * q:512 * (q + 1)],
                    start=True, stop=False, tile_position=(0, 32 * q),
                    skip_group_check=True)

        load_gx(0)
        if nwin > 1:
            load_gx(1)
        inject(0)
        for t in range(Tn):
            gc = gps[0]
            for k in range(KH):
                for q in range(4):
                    nc.tensor.matmul(
                        gc[32 * q:32 * (q + 1), :],
                        hT[0][:, Bl * k:Bl * (k + 1)],
                        wsb[:, k, 512 * q:512 * (q + 1)],
                        start=False, stop=(k == KH - 1),
                        tile_position=(0, 32 * q), skip_group_check=True)
            gact = wkpool.tile([128, H], BF16, tag="gact", name="gact")
            nc.scalar.activation(gact[0:96, :], gc[0:96, :], AF.Sigmoid)
            # i masked in place (base 0), f/o realigned+masked while tanh(g)
            # runs on ScalarE; mask=0 freezes h=c=0 (pad-packed semantics)
            gi0 = wkpool.tile([Bl, H], BF16, tag="gi0", name="gi0")
            nc.vector.tensor_scalar_mul(gi0[:], gact[0:32, :],
                                        mask_sb[:, t:t + 1])
            gf0 = wkpool.tile([Bl, H], BF16, tag="gf0", name="gf0")
            nc.vector.tensor_copy(gf0[:], gact[32:64, :])
            gf0m = wkpool.tile([Bl, H], BF16, tag="gf0m", name="gf0m")
            nc.vector.tensor_scalar_mul(gf0m[:], gf0[:], mask_sb[:, t:t + 1])
            gg0 = wkpool.tile([Bl, H], BF16, tag="gg0", name="gg0")
            nc.scalar.activation(gg0[:], gc[96:128, :], AF.Tanh)
            t1 = wkpool.tile([Bl, H], F32, tag="t1", name="t1")
            nc.vector.tensor_tensor(out=t1[:], in0=gf0m[:], in1=c_st[:],
                                    op=ALU.mult)
            t2 = wkpool.tile([Bl, H], BF16, tag="t2", name="t2")
            nc.vector.tensor_tensor(out=t2[:], in0=gi0[:], in1=gg0[:],
                                    op=ALU.mult)
            nc.vector.tensor_tensor(out=c_st[:], in0=t1[:], in1=t2[:],
                                    op=ALU.add)
            tch = wkpool.tile([Bl, H], BF16, tag="tch", name="tch")
            nc.scalar.activation(tch[:], c_st[:], AF.Tanh)
            go0 = wkpool.tile([Bl, H], BF16, tag="go0", name="go0")
            nc.vector.tensor_copy(go0[:], gact[64:96, :])
            go0m = wkpool.tile([Bl, H], BF16, tag="go0m", name="go0m")
            nc.vector.tensor_scalar_mul(go0m[:], go0[:], mask_sb[:, t:t + 1])
            h16 = wkpool.tile([Bl, H], BF16, tag="h16", name="h16")
            nc.vector.tensor_tensor(out=h16[:], in0=go0m[:], in1=tch[:],
                                    op=ALU.mult)
            hT_ps = tpspool.tile([128, KH * Bl], F32, tag="tps", name="hT_ps")
            if t + 1 < Tn:
                if (t + 1) % TC == 0 and (t + 1) // TC + 1 < nwin:
                    load_gx((t + 1) // TC + 1)
                inject(t + 1)
            hTn = htpool.tile([128, KH * Bl], BF16, tag="hT", name="hTn")
            for k in range(KH):
                nc.tensor.matmul(hT_ps[:, Bl * k:Bl * (k + 1)],
                                 h16[:, 128 * k:128 * (k + 1)], id_sb[:],
                                 start=True, stop=True, skip_group_check=True)
            nc.scalar.activation(hTn[:], hT_ps[:], AF.Copy)
            hT[0] = hTn
            nc.gpsimd.dma_start(hTv[:, t, :], hTn[:])


def _prep_inputs(inputs, Tn=T, Bl=BL):
    x = np.asarray(inputs["x"]).astype(np.int32)
    lengths = np.asarray(inputs["lengths"]).astype(np.int32)
    emb = np.asarray(inputs["emb"], dtype=np.float32)
    bf = ml_dtypes.bfloat16

    wt = {}
    for s in ("f1", "b1", "f2", "b2"):
        w_ih = np.asarray(inputs[f"W_ih_{s}"], np.float32)[_GATE_PERM]
        w_hh = np.asarray(inputs[f"W_hh_{s}"], np.float32)[_GATE_PERM]
        b = np.asarray(inputs[f"b_{s}"], np.float32)[_GATE_PERM]
        wt[f"wihT_{s}"] = np.ascontiguousarray(w_ih.T).astype(bf)
        wt[f"whhT_{s}"] = np.ascontiguousarray(w_hh.T).astype(bf)
        wt[f"bias_{s}"] = np.tile(b.reshape(1, G), (128, 1))
    com = {"emb": emb, "ident": np.eye(32, dtype=bf),
           "id128": np.eye(128, dtype=bf),
           "wclsT": np.ascontiguousarray(
               np.asarray(inputs["W_cls"], np.float32).T).astype(bf),
           "bcls": np.asarray(inputs["b_cls"], np.float32).reshape(TAGS, 1)}

    def chunked_timemajor(xscan):
        # v[tok] = xscan[b, t] with tok = t*32 + b  ->  idx[p, s] = v[128s+p]
        v = np.ascontiguousarray(xscan.T).reshape(-1)   # [t, b] flat
        return np.ascontiguousarray(v.reshape(-1, 128).T).astype(np.int32)

    ts = np.arange(Tn)[None, :]
    in_maps = [None] * NC
    for p in range(NPAIR):
        xs = x[Bl * p:Bl * (p + 1), :Tn]
        ls = np.minimum(lengths[Bl * p:Bl * (p + 1)], Tn)[:, None]
        for half, core in ((0, p), (1, p + NPAIR)):
            if half == 0:   # forward
                xscan = xs
                m = (ts < ls).astype(np.float32)
                sfx = ("f1", "f2")
            else:           # backward: global time flip + tail mask
                xscan = xs[:, ::-1]
                m = (ts >= Tn - ls).astype(np.float32)
                sfx = ("b1", "b2")
            fl = 1.0 if half == 0 else 0.0
            im = {"xg_idx": chunked_timemajor(xscan), "mask": m,
                  "flagF": np.full((128, 1), fl, np.float32),
                  "flagB": np.full((128, 1), 1.0 - fl, np.float32),
                  "wihT_l1": wt[f"wihT_{sfx[0]}"],
                  "whhT_l1": wt[f"whhT_{sfx[0]}"],
                  "bias_l1": wt[f"bias_{sfx[0]}"],
                  "wihT_l2": wt[f"wihT_{sfx[1]}"],
                  "whhT_l2": wt[f"whhT_{sfx[1]}"],
                  "bias_l2": wt[f"bias_{sfx[1]}"]}
            im.update(com)
            in_maps[core] = im
    return in_maps


_CACHED = {}


def kernel(**inputs) -> np.ndarray:
    if "nc" not in _CACHED:
        nc = bacc.Bacc("TRN2", target_bir_lowering=False, debug=False,
                       num_devices=NC)
        _build(nc)
        nc.compile()
        _CACHED["nc"] = nc
    nc = _CACHED["nc"]
    in_maps = _prep_inputs(inputs)
    res = run_bass_kernel_spmd(nc, in_maps, core_ids=list(range(NC)),
                               trace=False)
    out = np.empty((B, T, TAGS), np.float32)
    half_T = T // 2
    for p in range(NPAIR):
        for half, core in ((0, p), (1, p + NPAIR)):
            lt = res.results[core]["logitsT"]          # [TAGS, 8192]
            seq = lt.T.reshape(half_T, BL, TAGS)       # [t_scan, b, TAGS]
            seq = np.transpose(seq, (1, 0, 2))         # [b, t_scan, TAGS]
            if half == 0:   # fwd core: scan time = fwd time 0..256
                out[BL * p:BL * (p + 1), 0:half_T] = seq
            else:           # bwd core: scan steps 0..256 = fwd time 511..256
                out[BL * p:BL * (p + 1), half_T:T] = seq[:, ::-1]
    return out.astype(np.float32)


# revision 19
# speedup vs baseline: 1.7667x; 1.5097x over previous
"""BiLSTM tagger kernel, direction-parallel over 8 NeuronCores, v4.

Core c in 0..3 runs the FORWARD direction for sequences [32c, 32c+32);
core c+4 runs the BACKWARD direction for the same sequences. The backward
scan runs over GLOBALLY reversed time (host flips x along t); per-sequence
ragged reversal is handled by masking the i/f/o gates each step (mask=0
keeps h=c=0 through the pad region), which reproduces pad-packed semantics
exactly with no per-sequence index tensors.

Tokens are TIME-MAJOR (tok = t*32 + b) and all inter-phase tensors live in
TRANSPOSED layout hT[128, Tn*128] (col block t = 4 H-chunks x 32 batch),
written directly from the scan's per-step PE transpose. Backward cores
write their hT at column T-1-t, so every stored tensor is in forward time
order and the layer-2 projection + classifier read PLAIN contiguous
blocks: no indirect gathers and no DMA transposes anywhere. The backward
core's own layer-2 projection reads forward-time blocks as-is (inner
4-step order reversed) and its scan un-permutes by indexing the gx window
with 3-(t%4).

Scan: col-tiled quadrant psum layout - the 16 recurrent matmuls run as 4
rounds of 4 CONCURRENT col-tiled matmuls (tile_position=(0,32q)); psum
quadrant q = gate q (order i,f,o,g), sigmoid(i,f,o) is ONE [96,512] op.
All matmuls bf16, psum fp32.
"""

import sys

for _p in ("/opt/trn_rl_repo",):
    if _p not in sys.path:
        sys.path.append(_p)

import numpy as np
import ml_dtypes

import concourse.bass as bass
import concourse.tile as tile
from concourse import bacc, mybir
from concourse.bass import IndirectOffsetOnAxis
from concourse.bass_utils import run_bass_kernel_spmd

F32 = mybir.dt.float32
BF16 = mybir.dt.bfloat16
I32 = mybir.dt.int32
AF = mybir.ActivationFunctionType
ALU = mybir.AluOpType

B, T, V, E, H, TAGS = 128, 512, 50000, 256, 512, 64
NC = 8
NPAIR = NC // 2
BL = B // NPAIR          # 32 sequences per core (one direction each)
G = 4 * H
GROUPS = [[c, c + NPAIR] for c in range(NPAIR)]
NTOK = BL * T            # 16384 tokens per core, time-major: tok = t*32+b

# gate order i,f,o,g: psum quadrant q holds gate q, sigmoid covers 0:96
_GATE_PERM = np.concatenate([
    np.arange(0, H), np.arange(H, 2 * H), np.arange(3 * H, 4 * H),
    np.arange(2 * H, 3 * H)])


def _build(nc, Tn=T, Bl=BL):
    nchunk = NTOK // 128     # 128 token chunks (4 steps each)
    ncls = nchunk // 2       # 64 classifier chunks per core
    KE = E // 128            # 2
    KH = H // 128            # 4
    KH2 = 2 * H // 128       # 8

    emb = nc.dram_tensor("emb", [V, E], F32, kind="ExternalInput")
    xg_idx = nc.dram_tensor("xg_idx", [128, nchunk], I32, kind="ExternalInput")
    mask = nc.dram_tensor("mask", [Bl, Tn], F32, kind="ExternalInput")
    ident = nc.dram_tensor("ident", [32, 32], BF16, kind="ExternalInput")
    id128 = nc.dram_tensor("id128", [128, 128], BF16, kind="ExternalInput")
    flagF = nc.dram_tensor("flagF", [128, 1], F32, kind="ExternalInput")
    flagB = nc.dram_tensor("flagB", [128, 1], F32, kind="ExternalInput")

    wih, whh, biasd = {}, {}, {}
    for l, din in (("l1", E), ("l2", 2 * H)):
        wih[l] = nc.dram_tensor(f"wihT_{l}", [din, G], BF16, kind="ExternalInput")
        whh[l] = nc.dram_tensor(f"whhT_{l}", [H, G], BF16, kind="ExternalInput")
        biasd[l] = nc.dram_tensor(f"bias_{l}", [128, G], F32, kind="ExternalInput")
    wcls = nc.dram_tensor("wclsT", [2 * H, TAGS], BF16, kind="ExternalInput")
    bcls = nc.dram_tensor("bcls", [TAGS, 1], F32, kind="ExternalInput")

    gx = {l: nc.dram_tensor(f"gx_{l}", [NTOK, G], BF16) for l in ("l1", "l2")}
    hT = {l: nc.dram_tensor(f"hT_{l}", [128, Tn * 128], BF16)
          for l in ("l1", "l2")}
    hTp = {l: nc.dram_tensor(f"hTp_{l}", [2 * 128, Tn * 128], BF16)
           for l in ("l1", "l2")}
    logitsT = nc.dram_tensor("logitsT", [TAGS, NTOK // 2], F32,
                             kind="ExternalOutput")

    # Every core writes its hT in OWN-scan order (uniform). In the
    # exchanged buffer slot0 = fwd core's hT (columns = fwd time) and
    # slot1 = bwd core's (columns = bwd scan time = T-1-fwd). A core
    # reading x2 at its own scan step t needs the OWN slot natural and
    # the CROSS slot time-reversed; which slot is which depends on the
    # core, so both variants are loaded (cheap contiguous DMA) and
    # selected with a host 0/1 flag on the DVE (SPMD-uniform program).

    with tile.TileContext(nc) as tc:
        with tc.tile_pool(name="const", bufs=1) as cpool:
            def load_const(nm, shape, dt, src_ap):
                t = cpool.tile(shape, dt, name=nm, tag=nm)
                nc.gpsimd.dma_start(t[:], src_ap)
                return t

            xg_sb = load_const("xg_sb", [128, nchunk], I32, xg_idx[:])
            mask_sb = load_const("mask_sb", [Bl, Tn], F32, mask[:])
            id_sb = load_const("id_sb", [32, 32], BF16, ident[:])
            id128_sb = load_const("id128_sb", [128, 128], BF16, id128[:])
            bcls_sb = load_const("bcls_sb", [TAGS, 1], F32, bcls[:])
            fF_sb = load_const("fF_sb", [128, 1], F32, flagF[:])
            fB_sb = load_const("fB_sb", [128, 1], F32, flagB[:])
            bias_sb = {l: load_const(f"bias_sb_{l}", [128, G], F32, biasd[l][:])
                       for l in ("l1", "l2")}

            _proj1(nc, tc, nchunk, KE, wih["l1"], bias_sb["l1"], gx["l1"],
                   emb, xg_sb, id128_sb)
            _scan(nc, tc, Tn, Bl, KH, whh["l1"], gx["l1"], hT["l1"],
                  mask_sb, id_sb)
            nc.gpsimd.collective_compute(
                "AllGather", ALU.bypass, GROUPS,
                ins=[hT["l1"][:]], outs=[hTp["l1"][:]])
            _proj2(nc, tc, nchunk, Tn, wih["l2"], bias_sb["l2"], gx["l2"],
                   hTp["l1"], fF_sb, fB_sb)
            _scan(nc, tc, Tn, Bl, KH, whh["l2"], gx["l2"], hT["l2"],
                  mask_sb, id_sb)
            nc.gpsimd.collective_compute(
                "AllGather", ALU.bypass, GROUPS,
                ins=[hT["l2"][:]], outs=[hTp["l2"][:]])
            _classifier(nc, tc, ncls, Tn, wcls, bcls_sb, hTp["l2"], logitsT,
                        KH2, fF_sb, fB_sb)
    return nc


def _proj1(nc, tc, nchunk, KE, wih_d, bias_t, gx_d, emb, xg_sb, id128_sb):
    """gx1 = emb[x] @ W_ih1^T + b, time-major chunks of 128 tokens.
    Embedding rows gathered (fp32), cast, transposed on the PE."""
    with tc.tile_pool(name="pw", bufs=1) as wpool, \
         tc.tile_pool(name="pg", bufs=3) as gpool, \
         tc.tile_pool(name="pxps", bufs=2, space="PSUM") as xpspool, \
         tc.tile_pool(name="pps", bufs=4, space="PSUM") as ppool, \
         tc.tile_pool(name="pout", bufs=3) as opool:
        wsb = wpool.tile([128, KE, G], BF16, tag="w", name="wih1_sb")
        for k in range(KE):
            nc.gpsimd.dma_start(wsb[:, k, :], wih_d[128 * k:128 * (k + 1), :])
        for s in range(nchunk):
            e32 = gpool.tile([128, E], F32, tag="e32")
            nc.gpsimd.indirect_dma_start(
                out=e32[:], out_offset=None, in_=emb[:],
                in_offset=IndirectOffsetOnAxis(ap=xg_sb[:, s:s + 1], axis=0))
            e16 = gpool.tile([128, E], BF16, tag="e16")
            nc.vector.tensor_copy(e16[:], e32[:])
            xps = xpspool.tile([128, E], F32, tag="xps", name="xps")
            for kk in range(KE):
                nc.tensor.matmul(xps[:, 128 * kk:128 * (kk + 1)],
                                 e16[:, 128 * kk:128 * (kk + 1)], id128_sb[:],
                                 start=True, stop=True, skip_group_check=True)
            xT = gpool.tile([128, E], BF16, tag="xT")
            nc.scalar.activation(xT[:], xps[:], AF.Copy)
            gout = opool.tile([128, G], BF16, tag="gout")
            for n in range(4):
                ps = ppool.tile([128, 512], F32, tag="ps", name="pps")
                for kk in range(KE):
                    nc.tensor.matmul(
                        ps[:], xT[:, 128 * kk:128 * (kk + 1)],
                        wsb[:, kk, 512 * n:512 * (n + 1)],
                        start=(kk == 0), stop=(kk == KE - 1),
                        skip_group_check=True)
                nc.vector.tensor_tensor(
                    out=gout[:, 512 * n:512 * (n + 1)], in0=ps[:],
                    in1=bias_t[:, 512 * n:512 * (n + 1)], op=ALU.add)
            nc.gpsimd.dma_start(gx_d[128 * s:128 * (s + 1), :], gout[:])


def _load_x2(nc, xpool, hv, s, Tn, fF, fB, tag):
    """Load x2^T [128, 8, 4, 32] for chunk s (this core's scan steps
    4s..4s+4): slot d chunks k at rows 128d.. Both the natural and the
    time-reversed variant of each slot are loaded (contiguous DMA) and
    blended with the core's 0/1 flags: own slot natural, cross reversed.
    hv dims: [d, p, k, t, c32] (k before t so AP dim order matches dst)."""
    # tiles are t-major [128, d, t4, k4, c32] so each (slot, variant) is
    # ONE DMA: src AP dims (t, k, c) with strides (128, 32, 1); the
    # reversed variant just walks t backwards. Matmul lhsT for K-chunk kk
    # is the strided slice [:, d, :, kk&3, :].
    xn = xpool.tile([128, 2, 4, 4, 32], BF16, tag=tag + "n")
    xr = xpool.tile([128, 2, 4, 4, 32], BF16, tag=tag + "r")
    hi = Tn - 1 - 4 * s
    rsl = slice(hi, None, -1) if hi - 4 < 0 else slice(hi, hi - 4, -1)
    for d in range(2):
        nc.gpsimd.dma_start(xn[:, d, :, :, :], hv[d, :, 4 * s:4 * s + 4, :, :])
        nc.gpsimd.dma_start(xr[:, d, :, :, :], hv[d, :, rsl, :, :])
    # select per K-chunk so xT ends up K-MAJOR [p, d, k, t, c] with each
    # chunk's (t, c) contiguous - a legal one-free-dim matmul stationary.
    # slot0 (fwd dir): natural on fwd cores, reversed on bwd cores;
    # slot1 (bwd dir): the opposite.
    xT = xpool.tile([128, 2, 4, 4, 32], BF16, tag=tag)
    a = xpool.tile([128, 4, 32], BF16, tag=tag + "a")
    b = xpool.tile([128, 4, 32], BF16, tag=tag + "b")
    for d in range(2):
        fn, fr = (fF, fB) if d == 0 else (fB, fF)
        for k in range(4):
            nc.vector.tensor_scalar_mul(a[:], xn[:, d, :, k, :], fn[:, 0:1])
            nc.vector.tensor_scalar_mul(b[:], xr[:, d, :, k, :], fr[:, 0:1])
            nc.vector.tensor_tensor(out=xT[:, d, k, :, :], in0=a[:], in1=b[:],
                                    op=ALU.add)
    return xT


def _proj2(nc, tc, nchunk, Tn, wih_d, bias_t, gx_d, hTp_d, fF, fB):
    """gx2 = [out_f | out_b] @ W_ih2^T + b in this core's own scan order."""
    hv = hTp_d.ap().rearrange("(d p) (t k c) -> d p t k c", d=2, k=4, c=32)
    KD = 8
    with tc.tile_pool(name="qw", bufs=1) as wpool, \
         tc.tile_pool(name="qx", bufs=3) as xpool, \
         tc.tile_pool(name="qps", bufs=4, space="PSUM") as ppool, \
         tc.tile_pool(name="qout", bufs=3) as opool:
        wsb = wpool.tile([128, KD, G], BF16, tag="w", name="wih2_sb")
        for k in range(KD):
            nc.gpsimd.dma_start(wsb[:, k, :], wih_d[128 * k:128 * (k + 1), :])
        for s in range(nchunk):
            xT = _load_x2(nc, xpool, hv, s, Tn, fF, fB, "xT")
            gout = opool.tile([128, G], BF16, tag="gout")
            for n in range(4):
                ps = ppool.tile([128, 512], F32, tag="ps", name="qpps")
                for kk in range(KD):
                    nc.tensor.matmul(
                        ps[:], xT[:, kk // 4, kk % 4, :, :],
                        wsb[:, kk, 512 * n:512 * (n + 1)],
                        start=(kk == 0), stop=(kk == KD - 1),
                        skip_group_check=True)
                nc.vector.tensor_tensor(
                    out=gout[:, 512 * n:512 * (n + 1)], in0=ps[:],
                    in1=bias_t[:, 512 * n:512 * (n + 1)], op=ALU.add)
            nc.gpsimd.dma_start(gx_d[128 * s:128 * (s + 1), :], gout[:])


def _classifier(nc, tc, ncls, Tn, wcls_d, bcls_sb, hTp_d, logitsT, KH2,
                fF, fB):
    """logits for this core's half of the pair's tokens: chunks s=0..63 of
    its OWN scan time (host un-reverses bwd cores)."""
    hv = hTp_d.ap().rearrange("(d p) (t k c) -> d p t k c", d=2, k=4, c=32)
    with tc.tile_pool(name="cw", bufs=1) as wpool, \
         tc.tile_pool(name="cx", bufs=3) as xpool, \
         tc.tile_pool(name="cps", bufs=4, space="PSUM") as ppool, \
         tc.tile_pool(name="cout", bufs=3) as opool:
        wsb = wpool.tile([128, KH2, TAGS], BF16, tag="w", name="wcls_sb")
        for k in range(KH2):
            nc.gpsimd.dma_start(wsb[:, k, :], wcls_d[128 * k:128 * (k + 1), :])
        for s in range(ncls):
            o2T = _load_x2(nc, xpool, hv, s, Tn, fF, fB, "o2T")
            ps = ppool.tile([TAGS, 128], F32, tag="ps", name="cpps")
            for kk in range(KH2):
                nc.tensor.matmul(ps[:], wsb[:, kk, :],
                                 o2T[:, kk // 4, kk % 4, :, :],
                                 start=(kk == 0), stop=(kk == KH2 - 1),
                                 skip_group_check=True)
            lg = opool.tile([TAGS, 128], F32, tag="lg")
            nc.scalar.activation(lg[:], ps[:], AF.Identity,
                                 bias=bcls_sb[:, 0:1])
            nc.gpsimd.dma_start(logitsT[:, 128 * s:128 * (s + 1)], lg[:])


def _scan(nc, tc, Tn, Bl, KH, whh_d, gx_d, hTout_d, mask_sb, id_sb):
    """Single-direction scan, M=32, col-tiled quadrant psum layout.
    Gate masks (i,f,o multiplied by mask[:,t]) implement pad-packed
    semantics; the transposed state hTn is DMA'd per step straight into
    hTout (this core's scan order)."""
    TC = 4
    gxv = gx_d.ap().rearrange("(t b) d -> b t d", b=Bl)
    hTv = hTout_d.ap().rearrange("p (t c) -> p t c", c=128)
    with tc.tile_pool(name="sw", bufs=1) as wpool, \
         tc.tile_pool(name="sgx", bufs=3) as gxpool, \
         tc.tile_pool(name="sst", bufs=1) as stpool, \
         tc.tile_pool(name="sps", bufs=2, space="PSUM") as pspool, \
         tc.tile_pool(name="stps", bufs=2, space="PSUM") as tpspool, \
         tc.tile_pool(name="swk", bufs=3) as wkpool, \
         tc.tile_pool(name="shT", bufs=3) as htpool, \
         tc.tile_pool(name="srng", bufs=2) as rpool:
        wsb = wpool.tile([128, KH, G], BF16, tag="whh", name="whh_sb")
        for k in range(KH):
            nc.gpsimd.dma_start(wsb[:, k, :], whh_d[128 * k:128 * (k + 1), :])
        hT = [htpool.tile([128, KH * Bl], BF16, tag="hT", name="hT0")]
        nc.vector.memset(hT[0][:], 0.0)
        c_st = stpool.tile([Bl, H], F32, tag="c", name="c_st")
        nc.vector.memset(c_st[:], 0.0)
        gxc = {}
        gps = [None]
        ring = [None]
        nwin = (Tn + TC - 1) // TC

        def load_gx(w):
            tl = gxpool.tile([Bl, TC, G], BF16, tag="gx", name="gxc")
            nc.gpsimd.dma_start(tl[:], gxv[:, w * TC:(w + 1) * TC, :])
            gxc[w] = tl
            gxc.pop(w - 2, None)

        def inject(tt):
            gps[0] = pspool.tile([128, H], F32, tag="ps", name="gps")
            gxt = gxc[tt // TC]
            j = tt % TC
            for q in range(4):
                nc.tensor.matmul(
                    gps[0][32 * q:32 * (q + 1), :], id_sb[:],
                    gxt[:, j, 512 * q:512 * (q + 1)],
                    start=True, stop=False, tile_position=(0, 32 * q),
                    skip_group_check=True)

        load_gx(0)
        if nwin > 1:
            load_gx(1)
        inject(0)
        for t in range(Tn):
            gc = gps[0]
            for k in range(KH):
                for q in range(4):
                    nc.tensor.matmul(
                        gc[32 * q:32 * (q + 1), :],
                        hT[0][:, Bl * k:Bl * (k + 1)],
                        wsb[:, k, 512 * q:512 * (q + 1)],
                        start=False, stop=(k == KH - 1),
                        tile_position=(0, 32 * q), skip_group_check=True)
            gact = wkpool.tile([128, H], BF16, tag="gact", name="gact")
            nc.scalar.activation(gact[0:96, :], gc[0:96, :], AF.Sigmoid)
            # i masked in place (base 0), f/o realigned+masked while tanh(g)
            # runs on ScalarE; mask=0 freezes h=c=0 (pad-packed semantics)
            gi0 = wkpool.tile([Bl, H], BF16, tag="gi0", name="gi0")
            nc.vector.tensor_scalar_mul(gi0[:], gact[0:32, :],
                                        mask_sb[:, t:t + 1])
            gf0 = wkpool.tile([Bl, H], BF16, tag="gf0", name="gf0")
            nc.vector.tensor_copy(gf0[:], gact[32:64, :])
            gf0m = wkpool.tile([Bl, H], BF16, tag="gf0m", name="gf0m")
            nc.vector.tensor_scalar_mul(gf0m[:], gf0[:], mask_sb[:, t:t + 1])
            gg0 = wkpool.tile([Bl, H], BF16, tag="gg0", name="gg0")
            nc.scalar.activation(gg0[:], gc[96:128, :], AF.Tanh)
            t1 = wkpool.tile([Bl, H], F32, tag="t1", name="t1")
            nc.vector.tensor_tensor(out=t1[:], in0=gf0m[:], in1=c_st[:],
                                    op=ALU.mult)
            t2 = wkpool.tile([Bl, H], BF16, tag="t2", name="t2")
            nc.vector.tensor_tensor(out=t2[:], in0=gi0[:], in1=gg0[:],
                                    op=ALU.mult)
            nc.vector.tensor_tensor(out=c_st[:], in0=t1[:], in1=t2[:],
                                    op=ALU.add)
            tch = wkpool.tile([Bl, H], BF16, tag="tch", name="tch")
            nc.scalar.activation(tch[:], c_st[:], AF.Tanh)
            go0 = wkpool.tile([Bl, H], BF16, tag="go0", name="go0")
            nc.vector.tensor_copy(go0[:], gact[64:96, :])
            go0m = wkpool.tile([Bl, H], BF16, tag="go0m", name="go0m")
            nc.vector.tensor_scalar_mul(go0m[:], go0[:], mask_sb[:, t:t + 1])
            h16 = wkpool.tile([Bl, H], BF16, tag="h16", name="h16")
            nc.vector.tensor_tensor(out=h16[:], in0=go0m[:], in1=tch[:],
                                    op=ALU.mult)
            hT_ps = tpspool.tile([128, KH * Bl], F32, tag="tps", name="hT_ps")
            if t + 1 < Tn:
                if (t + 1) % TC == 0 and (t + 1) // TC + 1 < nwin:
                    load_gx((t + 1) // TC + 1)
                inject(t + 1)
            hTn = htpool.tile([128, KH * Bl], BF16, tag="hT", name="hTn")
            for k in range(KH):
                nc.tensor.matmul(hT_ps[:, Bl * k:Bl * (k + 1)],
                                 h16[:, 128 * k:128 * (k + 1)], id_sb[:],
                                 start=True, stop=True, skip_group_check=True)
            nc.scalar.activation(hTn[:], hT_ps[:], AF.Copy)
            hT[0] = hTn
            # ring-batch the hT writes: one [128, 4, 128] DMA per 4 steps
            if t % 4 == 0:
                ring[0] = rpool.tile([128, 4, 128], BF16, tag="rng",
                                     name="ring")
            nc.vector.tensor_copy(ring[0][:, t % 4, :], hTn[:])
            if (t + 1) % 4 == 0:
                nc.gpsimd.dma_start(hTv[:, t - 3:t + 1, :], ring[0][:])


def _prep_inputs(inputs, Tn=T, Bl=BL):
    x = np.asarray(inputs["x"]).astype(np.int32)
    lengths = np.asarray(inputs["lengths"]).astype(np.int32)
    emb = np.asarray(inputs["emb"], dtype=np.float32)
    bf = ml_dtypes.bfloat16

    wt = {}
    for s in ("f1", "b1", "f2", "b2"):
        w_ih = np.asarray(inputs[f"W_ih_{s}"], np.float32)[_GATE_PERM]
        w_hh = np.asarray(inputs[f"W_hh_{s}"], np.float32)[_GATE_PERM]
        b = np.asarray(inputs[f"b_{s}"], np.float32)[_GATE_PERM]
        wt[f"wihT_{s}"] = np.ascontiguousarray(w_ih.T).astype(bf)
        wt[f"whhT_{s}"] = np.ascontiguousarray(w_hh.T).astype(bf)
        wt[f"bias_{s}"] = np.tile(b.reshape(1, G), (128, 1))
    com = {"emb": emb, "ident": np.eye(32, dtype=bf),
           "id128": np.eye(128, dtype=bf),
           "wclsT": np.ascontiguousarray(
               np.asarray(inputs["W_cls"], np.float32).T).astype(bf),
           "bcls": np.asarray(inputs["b_cls"], np.float32).reshape(TAGS, 1)}

    def chunked_timemajor(xscan):
        # v[tok] = xscan[b, t] with tok = t*32 + b  ->  idx[p, s] = v[128s+p]
        v = np.ascontiguousarray(xscan.T).reshape(-1)   # [t, b] flat
        return np.ascontiguousarray(v.reshape(-1, 128).T).astype(np.int32)

    ts = np.arange(Tn)[None, :]
    in_maps = [None] * NC
    for p in range(NPAIR):
        xs = x[Bl * p:Bl * (p + 1), :Tn]
        ls = np.minimum(lengths[Bl * p:Bl * (p + 1)], Tn)[:, None]
        for half, core in ((0, p), (1, p + NPAIR)):
            if half == 0:   # forward
                xscan = xs
                m = (ts < ls).astype(np.float32)
                sfx = ("f1", "f2")
            else:           # backward: global time flip + tail mask
                xscan = xs[:, ::-1]
                m = (ts >= Tn - ls).astype(np.float32)
                sfx = ("b1", "b2")
            fl = 1.0 if half == 0 else 0.0
            im = {"xg_idx": chunked_timemajor(xscan), "mask": m,
                  "flagF": np.full((128, 1), fl, np.float32),
                  "flagB": np.full((128, 1), 1.0 - fl, np.float32),
                  "wihT_l1": wt[f"wihT_{sfx[0]}"],
                  "whhT_l1": wt[f"whhT_{sfx[0]}"],
                  "bias_l1": wt[f"bias_{sfx[0]}"],
                  "wihT_l2": wt[f"wihT_{sfx[1]}"],
                  "whhT_l2": wt[f"whhT_{sfx[1]}"],
                  "bias_l2": wt[f"bias_{sfx[1]}"]}
            im.update(com)
            in_maps[core] = im
    return in_maps


_CACHED = {}


def kernel(**inputs) -> np.ndarray:
    if "nc" not in _CACHED:
        nc = bacc.Bacc("TRN2", target_bir_lowering=False, debug=False,
                       num_devices=NC)
        _build(nc)
        nc.compile()
        _CACHED["nc"] = nc
    nc = _CACHED["nc"]
    in_maps = _prep_inputs(inputs)
    res = run_bass_kernel_spmd(nc, in_maps, core_ids=list(range(NC)),
                               trace=False)
    out = np.empty((B, T, TAGS), np.float32)
    half_T = T // 2
    for p in range(NPAIR):
        for half, core in ((0, p), (1, p + NPAIR)):
            lt = res.results[core]["logitsT"]          # [TAGS, 8192]
            seq = lt.T.reshape(half_T, BL, TAGS)       # [t_scan, b, TAGS]
            seq = np.transpose(seq, (1, 0, 2))         # [b, t_scan, TAGS]
            if half == 0:   # fwd core: scan time = fwd time 0..256
                out[BL * p:BL * (p + 1), 0:half_T] = seq
            else:           # bwd core: scan steps 0..256 = fwd time 511..256
                out[BL * p:BL * (p + 1), half_T:T] = seq[:, ::-1]
    return out.astype(np.float32)
